# revision 46
# baseline (speedup 1.0000x reference)
"""TP(heads)xDP(batch) sharded causal GQA attention block for 8 trn2 cores.

Each core c handles batch b=c//4 and head group g=c%4 (8 q heads, 2 kv heads).
Per-core pipeline (fused over 4 query chunks of 512):
  qkv = Wqkv_c @ x_b^T — chunk 0 in bf16; chunks 1-3 in fp8e4 DoubleRow
    (x and 64*Wqkv quantized to e4m3 host-side, 2 contraction tiles per
    matmul = 2x PE throughput; the 1/64 weight scale is undone in the psum
    evacuation; chunk 0 stays bf16 because early tokens have little softmax
    averaging to absorb quantization noise).
  RoPE on q/k (DVE + PE permutation matmul partition swap)
  scores_T[kj, qi] = k^T q  (bf16, 2 heads packed in the 128-row PE array)
  exp on ScalarE (no max subtraction), causal-masked on DVE for diag tiles
  out_aug[65, qi] = [v; ones]^T exp  (bf16; row 64 = softmax denominator)
  normalize via DVE reciprocal + gpsimd partition_broadcast
  partial_T[dout, t] = Wo_c^T attn  (bf16), host sums the 8 partials.
Scheduling: o_proj filler is pushed late (chunk1/2: o_proj(0) split,
chunk3: o_proj(1)+o_proj(2)) to feed the PE during the ACT-heavy late
chunks; the o_proj(3) tail uses the idle scores psum banks as extra po
accumulators and prefetches kt=0..2 so the in-order PE queue stays busy
(and the HAM clock-gate stays at 8/8) while the last pair normalizes.
"""
import sys
sys.path.insert(0, "/opt/trn_rl_repo")
from contextlib import ExitStack

import numpy as np
import ml_dtypes

E4 = ml_dtypes.float8_e4m3
B, L, D = 2, 2048, 2048
NH, NKV, HD = 32, 8, 64
ROPE_BASE = 10000.0
SCALE = HD ** -0.5
TC, TCW = 4, 512      # query/token chunks
NKD = 16              # d contraction tiles
NOT = 6               # output tiles per core (4 q packs, k pack, v pack)
NKJ = 16              # key tiles

F16 = ml_dtypes.bfloat16

_cached = {}


def _build_nc():
    import concourse.bacc as bacc
    import concourse.tile as tile
    import concourse.mybir as mybir
    from concourse import library_config

    F32 = mybir.dt.float32
    F32R = mybir.dt.float32r
    BF = mybir.dt.bfloat16
    F8 = mybir.dt.float8e4
    DR = mybir.MatmulPerfMode.DoubleRow
    AF = mybir.ActivationFunctionType

    nc = bacc.Bacc("TRN2", debug=False)
    xh_ap = nc.dram_tensor("xh", (TC, 128, NKD * TCW), BF, kind="ExternalInput").ap()
    xh8_ap = nc.dram_tensor("xh8", (TC, 128, NKD // 2, 2, TCW), F8,
                            kind="ExternalInput").ap()
    wq_ap = nc.dram_tensor("wq", (128, NOT * NKD * 128), BF, kind="ExternalInput").ap()
    wq8_ap = nc.dram_tensor("wq8", (128, NOT * NKD // 2, 2, 128), F8,
                            kind="ExternalInput").ap()
    wo_ap = nc.dram_tensor("wo", (128, 4 * NKD * 128), BF, kind="ExternalInput").ap()
    ccss_ap = nc.dram_tensor("ccss", (128, 2 * L), F32, kind="ExternalInput").ap()
    msk_ap = nc.dram_tensor("msk", (128, 4 * TCW), BF, kind="ExternalInput").ap()
    id2_ap = nc.dram_tensor("id2", (128, 64), F32R, kind="ExternalInput").ap()
    prm_ap = nc.dram_tensor("prm", (128, 128), BF, kind="ExternalInput").ap()
    out_ap = nc.dram_tensor("outp", (TC, NKD, 128, TCW), BF, kind="ExternalOutput").ap()

    with tile.TileContext(nc) as tcx, ExitStack() as ctx:
        pc = ctx.enter_context(tcx.tile_pool(name="const", bufs=1))
        px = ctx.enter_context(tcx.tile_pool(name="x", bufs=2))
        pw = ctx.enter_context(tcx.tile_pool(name="work", bufs=1))
        psc = ctx.enter_context(tcx.tile_pool(name="psc", bufs=2, space="PSUM"))
        paug = ctx.enter_context(tcx.tile_pool(name="paug", bufs=1, space="PSUM"))
        pmm = ctx.enter_context(tcx.tile_pool(name="pmm", bufs=2, space="PSUM"))

        wq_t = pc.tile([128, NOT * NKD * 128], BF)
        wo_t = pc.tile([128, 4 * NKD * 128], BF)
        ccss_t = pc.tile([128, 2 * L], F32)
        msk_t = pc.tile([128, 4 * TCW], BF)
        id2_t = pc.tile([128, 64], F32R)
        prm_t = pc.tile([128, 128], BF)
        wqw = NKD * 128
        # priority order: k-weights + rope tables first so chunk-0 rope/scores
        # start while the rest of the weights stream in.
        nc.sync.dma_start(wq_t[:, 4 * wqw:5 * wqw], wq_ap[:, 4 * wqw:5 * wqw])
        nc.sync.dma_start(ccss_t[:, 0:TCW], ccss_ap[:, 0:TCW])
        nc.sync.dma_start(ccss_t[:, L:L + TCW], ccss_ap[:, L:L + TCW])
        nc.sync.dma_start(prm_t[:, :], prm_ap[:, :])
        nc.sync.dma_start(wq_t[:, 5 * wqw:6 * wqw], wq_ap[:, 5 * wqw:6 * wqw])
        nc.sync.dma_start(id2_t[:, :], id2_ap[:, :])
        for ot in (0, 1, 2, 3):
            nc.sync.dma_start(wq_t[:, ot * wqw:(ot + 1) * wqw],
                              wq_ap[:, ot * wqw:(ot + 1) * wqw])
        nc.sync.dma_start(msk_t[:, :], msk_ap[:, :])
        nc.sync.dma_start(ccss_t[:, TCW:L], ccss_ap[:, TCW:L])
        nc.sync.dma_start(ccss_t[:, L + TCW:2 * L], ccss_ap[:, L + TCW:2 * L])
        wq8_t = pc.tile([128, NOT * NKD // 2, 2, 128], F8)
        for ot in (4, 5, 0, 1, 2, 3):
            nc.sync.dma_start(wq8_t[:, ot * 8:(ot + 1) * 8, :, :],
                              wq8_ap[:, ot * 8:(ot + 1) * 8, :, :])
        wow = 4 * NKD * 128 // 4
        for i in range(4):
            nc.sync.dma_start(wo_t[:, i * wow:(i + 1) * wow], wo_ap[:, i * wow:(i + 1) * wow])

        kpack = pc.tile([128, L], BF)        # k (2 kv heads stacked), RoPE'd
        vaug = pc.tile([128, 2 * NKJ * 65], BF)  # [v | ones] per (kv, kj)
        nc.vector.memset(vaug[:, 64::65], 1.0)   # ones columns

        with tcx.tile_critical():
            nc.gpsimd.load_library(library_config.attn)

        def rope_one(raw, tci, r):
            cs = ccss_t[:, tci * TCW:(tci + 1) * TCW]
            ss = ccss_t[:, L + tci * TCW:L + (tci + 1) * TCW]
            sl = slice(r * TCW, (r + 1) * TCW)
            swp = pmm.tile([128, TCW], F32, tag="mm")
            nc.tensor.matmul(swp[:, :], prm_t[:, :], raw[:, sl],
                             start=True, stop=True)
            with tcx.high_priority():
                nc.vector.tensor_mul(swp[:, :], swp[:, :], ss)
                nc.vector.tensor_mul(raw[:, sl], raw[:, sl], cs)
                nc.vector.tensor_add(raw[:, sl], raw[:, sl], swp[:, :])

        def rope_batched(raw, dest_ap, tci, nrep):
            """dest = raw*CC + swap32(raw)*SS; raw is fp16 [128, nrep*TCW] sbuf.
            Partition swap comes from a PE permutation matmul (prm_t)."""
            cs = ccss_t[:, tci * TCW:(tci + 1) * TCW]
            ss = ccss_t[:, L + tci * TCW:L + (tci + 1) * TCW]
            for r in range(nrep):
                sl = slice(r * TCW, (r + 1) * TCW)
                swp = pmm.tile([128, TCW], F32, tag="mm")
                nc.tensor.matmul(swp[:, :], prm_t[:, :], raw[:, sl],
                                 start=True, stop=True)
                with tcx.high_priority():
                    nc.vector.tensor_mul(swp[:, :], swp[:, :], ss)
                    nc.vector.tensor_mul(raw[:, sl], raw[:, sl], cs)
                    nc.vector.tensor_add(dest_ap[:, sl], raw[:, sl], swp[:, :])

        def o_proj_tiles(otc, dts, pairs_):
            for dt in dts:
                po = pmm.tile([128, TCW], F32, tag="mm")
                for kt in range(4):
                    nc.tensor.matmul(
                        po[:, :], wo_t[:, (kt * NKD + dt) * 128:(kt * NKD + dt + 1) * 128],
                        pairs_[kt][:, :],
                        start=(kt == 0), stop=(kt == 3))
                ev = pw.tile([128, TCW], BF, tag="ev", bufs=2)
                nc.vector.tensor_copy(ev[:, :], po[:, :])
                nc.sync.dma_start(out_ap[otc, dt], ev[:, :])

        def o_proj_tail(otc, pairs_):
            # tail-only: use the (now idle) scores psum tiles plus two pmm
            # tiles as po accumulators, and prefetch kt=0..2 for 6 dt before
            # any kt=3 matmul, so the (in-order) PE queue holds ~5us of work
            # while the last pack's pair normalization chain drains (else the
            # PE idles >3.4us, HAM rethrottles, and the tail runs cold).
            def wo_sl(kt, dt):
                return wo_t[:, (kt * NKD + dt) * 128:(kt * NKD + dt + 1) * 128]

            scs = [psc.tile([128, 2, TCW], F32, tag="scp", name=f"sct{g}")
                   for g in range(2)]
            pos = [pmm.tile([128, TCW], F32, tag="mm", name=f"pot{j}")
                   for j in range(2)]
            for kt in range(3):
                for g in range(2):
                    for j in range(2):
                        nc.tensor.matmul(scs[g][:, j, :], wo_sl(kt, 2 * g + j),
                                         pairs_[kt][:, :],
                                         start=(kt == 0), stop=False)
                for j in range(2):
                    nc.tensor.matmul(pos[j][:, :], wo_sl(kt, 4 + j),
                                     pairs_[kt][:, :],
                                     start=(kt == 0), stop=False)
            for g in range(2):
                for j in range(2):
                    nc.tensor.matmul(scs[g][:, j, :], wo_sl(3, 2 * g + j),
                                     pairs_[3][:, :], start=False, stop=True)
                ev2 = pw.tile([128, 2, TCW], BF, tag="ep", bufs=4)
                nc.vector.tensor_copy(ev2[:, :, :], scs[g][:, :, :])
                for j in range(2):
                    nc.sync.dma_start(out_ap[otc, 2 * g + j], ev2[:, j, :])
            for j in range(2):
                nc.tensor.matmul(pos[j][:, :], wo_sl(3, 4 + j),
                                 pairs_[3][:, :], start=False, stop=True)
                ev = pw.tile([128, TCW], BF, tag="ev", bufs=2)
                nc.vector.tensor_copy(ev[:, :], pos[j][:, :])
                nc.sync.dma_start(out_ap[otc, 4 + j], ev[:, :])
            for g in range(3, 8):
                sc = psc.tile([128, 2, TCW], F32, tag="scp")
                for j in range(2):
                    dt = 2 * g + j
                    for kt in range(4):
                        nc.tensor.matmul(sc[:, j, :], wo_sl(kt, dt),
                                         pairs_[kt][:, :],
                                         start=(kt == 0), stop=(kt == 3))
                ev2 = pw.tile([128, 2, TCW], BF, tag="ep", bufs=4)
                nc.vector.tensor_copy(ev2[:, :, :], sc[:, :, :])
                for j in range(2):
                    nc.sync.dma_start(out_ap[otc, 2 * g + j], ev2[:, j, :])

        def emit_xt_dma(tci):
            if tci == 0:
                xt = px.tile([128, NKD * TCW], BF, tag="xt", bufs=1)
                xw = NKD * TCW // 8
                with tcx.high_priority():
                    for i in range(8):
                        nc.sync.dma_start(xt[:, i * xw:(i + 1) * xw],
                                          xh_ap[tci][:, i * xw:(i + 1) * xw])
            else:
                xt = px.tile([128, NKD // 2, 2, TCW], F8, tag="xt8", bufs=2)
                with tcx.high_priority():
                    for i in range(8):
                        nc.sync.dma_start(xt[:, i, :, :], xh8_ap[tci][:, i, :, :])
            return xt

        def qkv_ot(tci, xt, qraw, ot):
            ps = pmm.tile([128, TCW], F32, tag="mm")
            if tci == 0:
                for dt in range(NKD):
                    nc.tensor.matmul(
                        ps[:, :], wq_t[:, (ot * NKD + dt) * 128:(ot * NKD + dt + 1) * 128],
                        xt[:, dt * TCW:(dt + 1) * TCW],
                        start=(dt == 0), stop=(dt == NKD - 1))
                rescale = None
            else:
                for m in range(NKD // 2):
                    nc.tensor.matmul(
                        ps[:, :], wq8_t[:, ot * 8 + m, :, :], xt[:, m, :, :],
                        start=(m == 0), stop=(m == NKD // 2 - 1), perf_mode=DR)
                rescale = 1.0 / 64.0

            def evac(dst, src):
                if rescale is None:
                    nc.vector.tensor_copy(dst, src)
                else:
                    nc.vector.tensor_scalar_mul(dst, src, rescale)

            if ot == 4:
                kraw = pw.tile([128, TCW], BF, tag="kraw", bufs=2)
                with tcx.high_priority():
                    evac(kraw[:, :], ps[:, :])
                rope_batched(kraw, kpack[:, tci * TCW:(tci + 1) * TCW], tci, 1)
            elif ot == 5:
                vch = pw.tile([128, TCW], F32R, tag="vch", bufs=2)
                with tcx.high_priority():
                    evac(vch[:, :], ps[:, :])
                for j in range(2):
                    for jj in range(4):
                        kj = 4 * tci + jj
                        tp = pmm.tile([128, 64], F32R, tag="mm")
                        nc.tensor.transpose(
                            tp[:, :], vch[64 * j:64 * j + 64, jj * 128:(jj + 1) * 128],
                            id2_t[64 * j:64 * j + 64, :])
                        col = (j * NKJ + kj) * 65
                        with tcx.high_priority():
                            nc.vector.tensor_copy(vaug[:, col:col + 64], tp[:, :])
            else:
                with tcx.high_priority():
                    evac(qraw[:, ot * TCW:(ot + 1) * TCW], ps[:, :])
                rope_one(qraw, tci, ot)

        # ---- prologue: chunk 0 qkv ----
        xt_cur = emit_xt_dma(0)
        qraw_cur = pw.tile([128, 4 * TCW], BF, tag="qraw", bufs=2)
        for ot in (4, 5, 0, 1, 2, 3):
            qkv_ot(0, xt_cur, qraw_cur, ot)

        QKV_SLOTS = ((4,), (5,), (0, 1), (2, 3))
        # o_proj placement: chunk -> {pack -> [(src_chunk, dts), ...]}
        OPROJ = {
            1: {p: [(0, range(2 * p, 2 * p + 2))] for p in range(4)},
            2: {p: [(0, range(8 + 2 * p, 8 + 2 * p + 2))] for p in range(4)},
            3: {p: [(1, range(4 * p, 4 * p + 4)),
                    (2, range(4 * p, 4 * p + 4))] for p in range(4)},
        }
        pairs_by_chunk = [[None] * 4 for _ in range(TC)]
        for tci in range(TC):
            qall = qraw_cur
            nxt = tci + 1
            if nxt < TC:
                xt_nxt = emit_xt_dma(nxt)
                qraw_nxt = pw.tile([128, 4 * TCW], BF, tag="qraw", bufs=2)

            last_kj = 4 * tci + 3
            for p in range(4):
                qs = slice(p * TCW, (p + 1) * TCW)
                augA = paug.tile([65, TCW], F32, tag="augA")
                augB = paug.tile([65, TCW], F32, tag="augB")
                # ---- key-tile loop (causal-trimmed, diag tiles masked) ----
                for kj in range(4 * tci + 4):
                    ks = slice(kj * 128, (kj + 1) * 128)
                    dj = kj - 4 * tci
                    qc0 = dj * 128 if dj > 0 else 0
                    qsA = slice(p * TCW + qc0, (p + 1) * TCW)
                    scp = psc.tile([128, 2, TCW], F32, tag="scp")
                    nc.tensor.matmul(scp[:, 0, qc0:TCW], kpack[0:64, ks], qall[0:64, qsA],
                                     start=True, stop=True, tile_position=(0, 0))
                    nc.tensor.matmul(scp[:, 1, qc0:TCW], kpack[64:128, ks], qall[64:128, qsA],
                                     start=True, stop=True, tile_position=(64, 0))
                    ep = pw.tile([128, 2, TCW], BF, tag="ep", bufs=4)
                    nc.scalar.activation(ep[:, :, qc0:TCW], scp[:, :, qc0:TCW],
                                         AF.Exp, scale=SCALE)
                    if dj >= 0:
                        mb = (msk_t[:, 0:128].unsqueeze(1)
                              .broadcast_to([128, 2, 128]))
                        with tcx.high_priority():
                            nc.vector.tensor_mul(ep[:, :, qc0:qc0 + 128],
                                                 ep[:, :, qc0:qc0 + 128], mb)
                    colA = kj * 65
                    colB = (NKJ + kj) * 65
                    nc.tensor.matmul(augA[:, qc0:TCW], vaug[:, colA:colA + 65],
                                     ep[:, 0, qc0:TCW],
                                     start=(kj == 0), stop=(kj == last_kj))
                    nc.tensor.matmul(augB[:, qc0:TCW], vaug[:, colB:colB + 65],
                                     ep[:, 1, qc0:TCW],
                                     start=(kj == 0), stop=(kj == last_kj))
                # ---- evacuate psum + normalize (gpsimd partition broadcast) ----
                augS = pw.tile([65, 2, TCW], F32, tag="augS", bufs=3)
                with tcx.high_priority():
                    nc.vector.tensor_copy(augS[:, 0, :], augA[:, :])
                    nc.vector.tensor_copy(augS[:, 1, :], augB[:, :])
                den2 = pw.tile([2, TCW], F32, tag="den2", bufs=4)
                nc.sync.dma_start(den2[:, :], augS[64:65, :, :])
                rc2 = pw.tile([2, TCW], F32, tag="rc2", bufs=4)
                nc.vector.reciprocal_approx_fast(rc2[:, :], den2[:, :])
                rcp = pw.tile([1, 2 * TCW], F32, tag="rcp", bufs=4)
                nc.sync.dma_start(rcp[:, :], rc2[:, :])
                pair = pw.tile([128, TCW], BF, tag="pair", bufs=12)
                bA = pw.tile([64, TCW], F32, tag="bA", bufs=2)
                nc.gpsimd.partition_broadcast(bA[:, :], rcp[0:1, 0:TCW])
                nc.vector.tensor_mul(pair[0:64, :], augS[0:64, 0, :], bA[:, :])
                bB = pw.tile([64, TCW], F32, tag="bB", bufs=2)
                nc.gpsimd.partition_broadcast(bB[:, :], rcp[0:1, TCW:2 * TCW])
                ob = pw.tile([64, TCW], BF, tag="ob", bufs=3)
                nc.vector.tensor_mul(ob[:, :], augS[0:64, 1, :], bB[:, :])
                nc.sync.dma_start(pair[64:128, :], ob[:, :])
                pairs_by_chunk[tci][p] = pair
                # ---- PE filler for the ACT-bound kj loop ----
                for otc, dts in OPROJ.get(tci, {}).get(p, ()):
                    o_proj_tiles(otc, dts, pairs_by_chunk[otc])
                if nxt < TC:
                    for ot in QKV_SLOTS[p]:
                        qkv_ot(nxt, xt_nxt, qraw_nxt, ot)

            if nxt < TC:
                qraw_cur = qraw_nxt
                xt_cur = xt_nxt

        o_proj_tail(TC - 1, pairs_by_chunk[TC - 1])

    nc.compile()
    return nc


def _host_prep(x, Wqkv, Wo):
    """Build per-core input maps. Returns list of 8 dicts."""
    invfreq = 1.0 / (ROPE_BASE ** (np.arange(0, HD, 2, dtype=np.float32) / HD))
    ang = np.arange(L, dtype=np.float32)[:, None] * invfreq[None, :]   # [L, 32]
    cos = np.cos(ang).T     # [32, L]
    sin = np.sin(ang).T
    cc = np.tile(cos, (4, 1)).astype(np.float32)                       # [128, L]
    sgn = np.repeat(np.array([-1.0, 1.0, -1.0, 1.0], np.float32), 32)
    ss = (np.tile(sin, (4, 1)) * sgn[:, None]).astype(np.float32)
    ccss = np.concatenate([cc, ss], axis=1)                            # [128, 2L]

    r = np.arange(128)[:, None]
    c = np.arange(TCW)[None, :]
    msk = np.concatenate(
        [(r + 128 * j <= c).astype(np.float32) for j in range(4)], axis=1
    ).astype(F16)                                                      # [128, 2048]

    id2 = np.zeros((128, 64), np.float32)
    id2[:64] = np.eye(64, dtype=np.float32)
    id2[64:] = np.eye(64, dtype=np.float32)

    prm = np.zeros((128, 128), np.float32)
    prm[np.arange(128), np.arange(128) ^ 32] = 1.0                     # swap32 perm
    prm = prm.astype(F16)

    wq_part = Wqkv[:NH * HD].reshape(NH, HD, D)
    wk_part = Wqkv[NH * HD:NH * HD + NKV * HD].reshape(NKV, HD, D)
    wv_part = Wqkv[NH * HD + NKV * HD:].reshape(NKV, HD, D)

    in_maps = []
    for core in range(8):
        b, g = core // 4, core % 4
        xT = np.ascontiguousarray(x[b].T)                              # [D, L]
        xh_f = (xT.reshape(NKD, 128, TC, TCW).transpose(2, 1, 0, 3)
                .reshape(TC, 128, NKD * TCW))
        xh = xh_f.astype(F16)
        xh8 = (np.clip(xh_f, -240, 240)
               .reshape(TC, 128, NKD // 2, 2, TCW).astype(E4))

        rows = []
        for p in range(4):
            rows.append(wq_part[8 * g + p])
            rows.append(wq_part[8 * g + 4 + p])
        rows.append(wk_part[2 * g]); rows.append(wk_part[2 * g + 1])
        rows.append(wv_part[2 * g]); rows.append(wv_part[2 * g + 1])
        Wc = np.concatenate(rows, axis=0)                              # [768, D]
        wq_f = (Wc.reshape(NOT, 128, NKD, 128).transpose(3, 0, 2, 1)
                .reshape(128, NOT * NKD * 128))
        wq = wq_f.astype(F16)
        wq8 = (np.clip(wq_f * 64.0, -240, 240)
               .reshape(128, NOT * NKD // 2, 2, 128).astype(E4))

        cols = np.empty((4, 128), np.int64)
        for kt in range(4):
            cols[kt, :64] = (8 * g + kt) * HD + np.arange(64)
            cols[kt, 64:] = (8 * g + 4 + kt) * HD + np.arange(64)
        Woc = Wo.T[cols.reshape(-1)]                                   # [512, D]
        wo = (Woc.reshape(4, 128, NKD, 128).transpose(1, 0, 2, 3)
              .reshape(128, 4 * NKD * 128)).astype(F16)

        in_maps.append(dict(xh=xh, xh8=xh8, wq=wq, wq8=wq8, wo=wo,
                            ccss=ccss, msk=msk, id2=id2, prm=prm))
    return in_maps


def _get_nc():
    if "nc" not in _cached:
        _cached["nc"] = _build_nc()
    return _cached["nc"]


def run_sharded(x, Wqkv, Wo, trace=False):
    """Run on 8 cores; returns (out [B,L,D] float32, BassKernelResults)."""
    from concourse.bass_utils import run_bass_kernel_spmd
    nc = _get_nc()
    in_maps = _host_prep(np.asarray(x, np.float32), np.asarray(Wqkv, np.float32),
                         np.asarray(Wo, np.float32))
    res = run_bass_kernel_spmd(nc, in_maps, list(range(8)), trace=trace)
    out = np.zeros((B, L, D), np.float64)
    for core in range(8):
        b = core // 4
        P = res.results[core]["outp"].transpose(1, 2, 0, 3).reshape(D, L)
        out[b] += P.T.astype(np.float64)
    return out.astype(np.float32), res


def kernel(x, Wqkv, Wo):
    out, _ = run_sharded(x, Wqkv, Wo, trace=False)
    return out


# revision 50
# speedup vs baseline: 1.0229x; 1.0229x over previous
"""TP(heads)xDP(batch) sharded causal GQA attention block for 8 trn2 cores.

Each core c handles batch b=c//4 and head group g=c%4 (8 q heads, 2 kv heads).
Per-core pipeline (fused over 4 query chunks of 512):
  qkv = Wqkv_c @ x_b^T — chunk 0 in bf16; chunks 1-3 in fp8e4 DoubleRow
    (x and 64*Wqkv quantized to e4m3 host-side, 2 contraction tiles per
    matmul = 2x PE throughput; the 1/64 weight scale is undone in the psum
    evacuation; chunk 0 stays bf16 because early tokens have little softmax
    averaging to absorb quantization noise).
  RoPE on q/k (DVE + PE permutation matmul partition swap)
  scores_T[kj, qi] = k^T q  (bf16, 2 heads packed in the 128-row PE array)
  exp on ScalarE (no max subtraction), causal-masked on DVE for diag tiles
  out_aug[65, qi] = [v; ones]^T exp  (bf16; row 64 = softmax denominator)
  normalize via DVE reciprocal + gpsimd partition_broadcast
  partial_T[dout, t] = Wo_c^T attn  (bf16), host sums the 8 partials.
Scheduling: o_proj filler is pushed late (chunk1/2: o_proj(0) split,
chunk3: o_proj(1)+o_proj(2)) to feed the PE during the ACT-heavy late
chunks; the o_proj(3) tail uses the idle scores psum banks as extra po
accumulators and prefetches kt=0..2 so the in-order PE queue stays busy
(and the HAM clock-gate stays at 8/8) while the last pair normalizes.
"""
import sys
sys.path.insert(0, "/opt/trn_rl_repo")
from contextlib import ExitStack

import numpy as np
import ml_dtypes

E4 = ml_dtypes.float8_e4m3
B, L, D = 2, 2048, 2048
NH, NKV, HD = 32, 8, 64
ROPE_BASE = 10000.0
SCALE = HD ** -0.5
TC, TCW = 4, 512      # query/token chunks
NKD = 16              # d contraction tiles
NOT = 6               # output tiles per core (4 q packs, k pack, v pack)
NKJ = 16              # key tiles

F16 = ml_dtypes.bfloat16

_cached = {}


def _build_nc():
    import concourse.bacc as bacc
    import concourse.tile as tile
    import concourse.mybir as mybir
    from concourse import library_config

    F32 = mybir.dt.float32
    F32R = mybir.dt.float32r
    BF = mybir.dt.bfloat16
    F8 = mybir.dt.float8e4
    DR = mybir.MatmulPerfMode.DoubleRow
    AF = mybir.ActivationFunctionType

    nc = bacc.Bacc("TRN2", debug=False)
    xh_ap = nc.dram_tensor("xh", (TC, 128, NKD * TCW), BF, kind="ExternalInput").ap()
    xh8_ap = nc.dram_tensor("xh8", (TC, 128, NKD // 2, 2, TCW), F8,
                            kind="ExternalInput").ap()
    wq_ap = nc.dram_tensor("wq", (128, NOT * NKD * 128), BF, kind="ExternalInput").ap()
    wq8_ap = nc.dram_tensor("wq8", (128, NOT * NKD // 2, 2, 128), F8,
                            kind="ExternalInput").ap()
    wo_ap = nc.dram_tensor("wo", (128, 4 * NKD * 128), BF, kind="ExternalInput").ap()
    ccss_ap = nc.dram_tensor("ccss", (128, 2 * L), F32, kind="ExternalInput").ap()
    msk_ap = nc.dram_tensor("msk", (128, 4 * TCW), BF, kind="ExternalInput").ap()
    id2_ap = nc.dram_tensor("id2", (128, 64), F32R, kind="ExternalInput").ap()
    prm_ap = nc.dram_tensor("prm", (128, 128), BF, kind="ExternalInput").ap()
    out_ap = nc.dram_tensor("outp", (TC, NKD, 128, TCW), BF, kind="ExternalOutput").ap()

    with tile.TileContext(nc) as tcx, ExitStack() as ctx:
        pc = ctx.enter_context(tcx.tile_pool(name="const", bufs=1))
        px = ctx.enter_context(tcx.tile_pool(name="x", bufs=2))
        pw = ctx.enter_context(tcx.tile_pool(name="work", bufs=1))
        psc = ctx.enter_context(tcx.tile_pool(name="psc", bufs=2, space="PSUM"))
        paug = ctx.enter_context(tcx.tile_pool(name="paug", bufs=1, space="PSUM"))
        pmm = ctx.enter_context(tcx.tile_pool(name="pmm", bufs=2, space="PSUM"))

        wq_t = pc.tile([128, NOT * NKD * 128], BF)
        wo_t = pc.tile([128, 4 * NKD * 128], BF)
        ccss_t = pc.tile([128, 2 * L], F32)
        msk_t = pc.tile([128, 4 * TCW], BF)
        id2_t = pc.tile([128, 64], F32R)
        prm_t = pc.tile([128, 128], BF)
        wqw = NKD * 128
        # priority order: k-weights + rope tables first so chunk-0 rope/scores
        # start while the rest of the weights stream in.
        nc.sync.dma_start(wq_t[:, 4 * wqw:5 * wqw], wq_ap[:, 4 * wqw:5 * wqw])
        nc.sync.dma_start(ccss_t[:, 0:TCW], ccss_ap[:, 0:TCW])
        nc.sync.dma_start(ccss_t[:, L:L + TCW], ccss_ap[:, L:L + TCW])
        nc.sync.dma_start(prm_t[:, :], prm_ap[:, :])
        nc.sync.dma_start(wq_t[:, 5 * wqw:6 * wqw], wq_ap[:, 5 * wqw:6 * wqw])
        nc.sync.dma_start(id2_t[:, :], id2_ap[:, :])
        for ot in (0, 1, 2, 3):
            nc.sync.dma_start(wq_t[:, ot * wqw:(ot + 1) * wqw],
                              wq_ap[:, ot * wqw:(ot + 1) * wqw])
        nc.sync.dma_start(msk_t[:, :], msk_ap[:, :])
        nc.sync.dma_start(ccss_t[:, TCW:L], ccss_ap[:, TCW:L])
        nc.sync.dma_start(ccss_t[:, L + TCW:2 * L], ccss_ap[:, L + TCW:2 * L])
        wq8_t = pc.tile([128, NOT * NKD // 2, 2, 128], F8)
        for ot in (4, 5, 0, 1, 2, 3):
            nc.sync.dma_start(wq8_t[:, ot * 8:(ot + 1) * 8, :, :],
                              wq8_ap[:, ot * 8:(ot + 1) * 8, :, :])
        wow = 4 * NKD * 128 // 4
        for i in range(4):
            nc.sync.dma_start(wo_t[:, i * wow:(i + 1) * wow], wo_ap[:, i * wow:(i + 1) * wow])

        kpack = pc.tile([128, L], BF)        # k (2 kv heads stacked), RoPE'd
        vaug = pc.tile([128, 2 * NKJ * 65], BF)  # [v | ones] per (kv, kj)
        nc.vector.memset(vaug[:, 64::65], 1.0)   # ones columns

        with tcx.tile_critical():
            nc.gpsimd.load_library(library_config.attn)

        def rope_one(raw, tci, r):
            cs = ccss_t[:, tci * TCW:(tci + 1) * TCW]
            ss = ccss_t[:, L + tci * TCW:L + (tci + 1) * TCW]
            sl = slice(r * TCW, (r + 1) * TCW)
            swp = pmm.tile([128, TCW], F32, tag="mm")
            nc.tensor.matmul(swp[:, :], prm_t[:, :], raw[:, sl],
                             start=True, stop=True)
            with tcx.high_priority():
                nc.vector.tensor_mul(swp[:, :], swp[:, :], ss)
                nc.vector.tensor_mul(raw[:, sl], raw[:, sl], cs)
                nc.vector.tensor_add(raw[:, sl], raw[:, sl], swp[:, :])

        def rope_batched(raw, dest_ap, tci, nrep):
            """dest = raw*CC + swap32(raw)*SS; raw is fp16 [128, nrep*TCW] sbuf.
            Partition swap comes from a PE permutation matmul (prm_t)."""
            cs = ccss_t[:, tci * TCW:(tci + 1) * TCW]
            ss = ccss_t[:, L + tci * TCW:L + (tci + 1) * TCW]
            for r in range(nrep):
                sl = slice(r * TCW, (r + 1) * TCW)
                swp = pmm.tile([128, TCW], F32, tag="mm")
                nc.tensor.matmul(swp[:, :], prm_t[:, :], raw[:, sl],
                                 start=True, stop=True)
                with tcx.high_priority():
                    nc.vector.tensor_mul(swp[:, :], swp[:, :], ss)
                    nc.vector.tensor_mul(raw[:, sl], raw[:, sl], cs)
                    nc.vector.tensor_add(dest_ap[:, sl], raw[:, sl], swp[:, :])

        def o_proj_tiles(otc, dts, pairs_):
            for dt in dts:
                po = pmm.tile([128, TCW], F32, tag="mm")
                for kt in range(4):
                    nc.tensor.matmul(
                        po[:, :], wo_t[:, (kt * NKD + dt) * 128:(kt * NKD + dt + 1) * 128],
                        pairs_[kt][:, :],
                        start=(kt == 0), stop=(kt == 3))
                ev = pw.tile([128, TCW], BF, tag="ev", bufs=2)
                nc.vector.tensor_copy(ev[:, :], po[:, :])
                nc.sync.dma_start(out_ap[otc, dt], ev[:, :])

        def o_proj_tail(otc, pairs_):
            # tail-only: use the (now idle) scores psum tiles plus two pmm
            # tiles as po accumulators, and prefetch kt=0..2 for 6 dt before
            # any kt=3 matmul, so the (in-order) PE queue holds ~5us of work
            # while the last pack's pair normalization chain drains (else the
            # PE idles >3.4us, HAM rethrottles, and the tail runs cold).
            def wo_sl(kt, dt):
                return wo_t[:, (kt * NKD + dt) * 128:(kt * NKD + dt + 1) * 128]

            scs = [psc.tile([128, 2, TCW], F32, tag="scp", name=f"sct{g}")
                   for g in range(2)]
            pos = [pmm.tile([128, TCW], F32, tag="mm", name=f"pot{j}")
                   for j in range(2)]
            for kt in range(3):
                for g in range(2):
                    for j in range(2):
                        nc.tensor.matmul(scs[g][:, j, :], wo_sl(kt, 2 * g + j),
                                         pairs_[kt][:, :],
                                         start=(kt == 0), stop=False)
                for j in range(2):
                    nc.tensor.matmul(pos[j][:, :], wo_sl(kt, 4 + j),
                                     pairs_[kt][:, :],
                                     start=(kt == 0), stop=False)
            for g in range(2):
                for j in range(2):
                    nc.tensor.matmul(scs[g][:, j, :], wo_sl(3, 2 * g + j),
                                     pairs_[3][:, :], start=False, stop=True)
                ev2 = pw.tile([128, 2, TCW], BF, tag="ep", bufs=4)
                nc.vector.tensor_copy(ev2[:, :, :], scs[g][:, :, :])
                for j in range(2):
                    nc.sync.dma_start(out_ap[otc, 2 * g + j], ev2[:, j, :])
            for j in range(2):
                nc.tensor.matmul(pos[j][:, :], wo_sl(3, 4 + j),
                                 pairs_[3][:, :], start=False, stop=True)
                ev = pw.tile([128, TCW], BF, tag="ev", bufs=2)
                nc.vector.tensor_copy(ev[:, :], pos[j][:, :])
                nc.sync.dma_start(out_ap[otc, 4 + j], ev[:, :])
            for g in range(3, 8):
                sc = psc.tile([128, 2, TCW], F32, tag="scp")
                for j in range(2):
                    dt = 2 * g + j
                    for kt in range(4):
                        nc.tensor.matmul(sc[:, j, :], wo_sl(kt, dt),
                                         pairs_[kt][:, :],
                                         start=(kt == 0), stop=(kt == 3))
                ev2 = pw.tile([128, 2, TCW], BF, tag="ep", bufs=4)
                nc.vector.tensor_copy(ev2[:, :, :], sc[:, :, :])
                for j in range(2):
                    nc.sync.dma_start(out_ap[otc, 2 * g + j], ev2[:, j, :])

        def emit_xt_dma(tci):
            if tci == 0:
                xt = px.tile([128, NKD * TCW], BF, tag="xt", bufs=1)
                xw = NKD * TCW // 8
                with tcx.high_priority():
                    for i in range(8):
                        nc.sync.dma_start(xt[:, i * xw:(i + 1) * xw],
                                          xh_ap[tci][:, i * xw:(i + 1) * xw])
            else:
                xt = px.tile([128, NKD // 2, 2, TCW], F8, tag="xt8", bufs=2)
                with tcx.high_priority():
                    for i in range(8):
                        nc.sync.dma_start(xt[:, i, :, :], xh8_ap[tci][:, i, :, :])
            return xt

        def qkv_ot(tci, xt, qraw, ot):
            ps = pmm.tile([128, TCW], F32, tag="mm")
            if tci == 0:
                for dt in range(NKD):
                    nc.tensor.matmul(
                        ps[:, :], wq_t[:, (ot * NKD + dt) * 128:(ot * NKD + dt + 1) * 128],
                        xt[:, dt * TCW:(dt + 1) * TCW],
                        start=(dt == 0), stop=(dt == NKD - 1))
                rescale = None
            else:
                for m in range(NKD // 2):
                    nc.tensor.matmul(
                        ps[:, :], wq8_t[:, ot * 8 + m, :, :], xt[:, m, :, :],
                        start=(m == 0), stop=(m == NKD // 2 - 1), perf_mode=DR)
                rescale = 1.0 / 64.0

            def evac(dst, src):
                if rescale is None:
                    nc.vector.tensor_copy(dst, src)
                else:
                    nc.vector.tensor_scalar_mul(dst, src, rescale)

            if ot == 4:
                kraw = pw.tile([128, TCW], BF, tag="kraw", bufs=2)
                with tcx.high_priority():
                    evac(kraw[:, :], ps[:, :])
                rope_batched(kraw, kpack[:, tci * TCW:(tci + 1) * TCW], tci, 1)
            elif ot == 5:
                vch = pw.tile([128, TCW], F32R, tag="vch", bufs=2)
                with tcx.high_priority():
                    evac(vch[:, :], ps[:, :])
                for j in range(2):
                    for jj in range(4):
                        kj = 4 * tci + jj
                        tp = pmm.tile([128, 64], F32R, tag="mm")
                        nc.tensor.transpose(
                            tp[:, :], vch[64 * j:64 * j + 64, jj * 128:(jj + 1) * 128],
                            id2_t[64 * j:64 * j + 64, :])
                        col = (j * NKJ + kj) * 65
                        with tcx.high_priority():
                            nc.vector.tensor_copy(vaug[:, col:col + 64], tp[:, :])
            else:
                with tcx.high_priority():
                    evac(qraw[:, ot * TCW:(ot + 1) * TCW], ps[:, :])
                rope_one(qraw, tci, ot)

        # ---- prologue: chunk 0 qkv ----
        xt_cur = emit_xt_dma(0)
        qraw_cur = pw.tile([128, 4 * TCW], BF, tag="qraw", bufs=2)
        for ot in (4, 5, 0, 1, 2, 3):
            qkv_ot(0, xt_cur, qraw_cur, ot)

        QKV_SLOTS = ((4,), (5,), (0, 1), (2, 3))
        # o_proj placement: chunk -> {pack -> [(src_chunk, dts), ...]}
        OPROJ = {
            1: {p: [(0, range(2 * p, 2 * p + 2))] for p in range(4)},
            2: {p: [(0, range(8 + 2 * p, 8 + 2 * p + 2))] for p in range(4)},
            3: {p: [(1, range(4 * p, 4 * p + 4)),
                    (2, range(4 * p, 4 * p + 4))] for p in range(4)},
        }
        pairs_by_chunk = [[None] * 4 for _ in range(TC)]
        for tci in range(TC):
            qall = qraw_cur
            nxt = tci + 1
            if nxt < TC:
                xt_nxt = emit_xt_dma(nxt)
                qraw_nxt = pw.tile([128, 4 * TCW], BF, tag="qraw", bufs=2)

            last_kj = 4 * tci + 3
            for p in range(4):
                qs = slice(p * TCW, (p + 1) * TCW)
                augA = paug.tile([65, TCW], F32, tag="augA")
                augB = paug.tile([65, TCW], F32, tag="augB")
                # ---- key-tile loop (causal-trimmed, diag tiles masked) ----
                for kj in range(4 * tci + 4):
                    ks = slice(kj * 128, (kj + 1) * 128)
                    dj = kj - 4 * tci
                    qc0 = dj * 128 if dj > 0 else 0
                    qsA = slice(p * TCW + qc0, (p + 1) * TCW)
                    scp = psc.tile([128, 2, TCW], F32, tag="scp")
                    nc.tensor.matmul(scp[:, 0, qc0:TCW], kpack[0:64, ks], qall[0:64, qsA],
                                     start=True, stop=True, tile_position=(0, 0))
                    nc.tensor.matmul(scp[:, 1, qc0:TCW], kpack[64:128, ks], qall[64:128, qsA],
                                     start=True, stop=True, tile_position=(64, 0))
                    ep = pw.tile([128, 2, TCW], BF, tag="ep", bufs=4)
                    nc.scalar.activation(ep[:, :, qc0:TCW], scp[:, :, qc0:TCW],
                                         AF.Exp, scale=SCALE)
                    if dj >= 0:
                        mb = (msk_t[:, 0:128].unsqueeze(1)
                              .broadcast_to([128, 2, 128]))
                        with tcx.high_priority():
                            nc.vector.tensor_mul(ep[:, :, qc0:qc0 + 128],
                                                 ep[:, :, qc0:qc0 + 128], mb)
                    colA = kj * 65
                    colB = (NKJ + kj) * 65
                    nc.tensor.matmul(augA[:, qc0:TCW], vaug[:, colA:colA + 65],
                                     ep[:, 0, qc0:TCW],
                                     start=(kj == 0), stop=(kj == last_kj))
                    nc.tensor.matmul(augB[:, qc0:TCW], vaug[:, colB:colB + 65],
                                     ep[:, 1, qc0:TCW],
                                     start=(kj == 0), stop=(kj == last_kj))
                # ---- evacuate psum + normalize (gpsimd partition broadcast) ----
                augS = pw.tile([65, 2, TCW], F32, tag="augS", bufs=3)
                with tcx.high_priority():
                    nc.vector.tensor_copy(augS[:, 0, :], augA[:, :])
                    nc.vector.tensor_copy(augS[:, 1, :], augB[:, :])
                den2 = pw.tile([2, TCW], F32, tag="den2", bufs=4)
                nc.sync.dma_start(den2[:, :], augS[64:65, :, :])
                rc2 = pw.tile([2, TCW], F32, tag="rc2", bufs=4)
                nc.vector.reciprocal_approx_fast(rc2[:, :], den2[:, :])
                rcp = pw.tile([1, 2 * TCW], F32, tag="rcp", bufs=4)
                nc.sync.dma_start(rcp[:, :], rc2[:, :])
                pair = pw.tile([128, TCW], BF, tag="pair", bufs=12)
                bA = pw.tile([64, TCW], F32, tag="bA", bufs=2)
                nc.gpsimd.partition_broadcast(bA[:, :], rcp[0:1, 0:TCW])
                nc.vector.tensor_mul(pair[0:64, :], augS[0:64, 0, :], bA[:, :])
                bB = pw.tile([64, TCW], F32, tag="bB", bufs=2)
                nc.gpsimd.partition_broadcast(bB[:, :], rcp[0:1, TCW:2 * TCW])
                ob = pw.tile([64, TCW], BF, tag="ob", bufs=3)
                nc.vector.tensor_mul(ob[:, :], augS[0:64, 1, :], bB[:, :])
                nc.sync.dma_start(pair[64:128, :], ob[:, :])
                pairs_by_chunk[tci][p] = pair
                # ---- PE filler for the ACT-bound kj loop ----
                for otc, dts in OPROJ.get(tci, {}).get(p, ()):
                    o_proj_tiles(otc, dts, pairs_by_chunk[otc])
                if nxt < TC:
                    for ot in QKV_SLOTS[p]:
                        qkv_ot(nxt, xt_nxt, qraw_nxt, ot)

            if nxt < TC:
                qraw_cur = qraw_nxt
                xt_cur = xt_nxt

        o_proj_tail(TC - 1, pairs_by_chunk[TC - 1])

    nc.compile()
    return nc


def _host_prep(x, Wqkv, Wo):
    """Build per-core input maps. Returns list of 8 dicts."""
    invfreq = 1.0 / (ROPE_BASE ** (np.arange(0, HD, 2, dtype=np.float32) / HD))
    ang = np.arange(L, dtype=np.float32)[:, None] * invfreq[None, :]   # [L, 32]
    cos = np.cos(ang).T     # [32, L]
    sin = np.sin(ang).T
    cc = np.tile(cos, (4, 1)).astype(np.float32)                       # [128, L]
    sgn = np.repeat(np.array([-1.0, 1.0, -1.0, 1.0], np.float32), 32)
    ss = (np.tile(sin, (4, 1)) * sgn[:, None]).astype(np.float32)
    ccss = np.concatenate([cc, ss], axis=1)                            # [128, 2L]

    r = np.arange(128)[:, None]
    c = np.arange(TCW)[None, :]
    msk = np.concatenate(
        [(r + 128 * j <= c).astype(np.float32) for j in range(4)], axis=1
    ).astype(F16)                                                      # [128, 2048]

    id2 = np.zeros((128, 64), np.float32)
    id2[:64] = np.eye(64, dtype=np.float32)
    id2[64:] = np.eye(64, dtype=np.float32)

    prm = np.zeros((128, 128), np.float32)
    prm[np.arange(128), np.arange(128) ^ 32] = 1.0                     # swap32 perm
    prm = prm.astype(F16)

    wq_part = Wqkv[:NH * HD].reshape(NH, HD, D)
    wk_part = Wqkv[NH * HD:NH * HD + NKV * HD].reshape(NKV, HD, D)
    wv_part = Wqkv[NH * HD + NKV * HD:].reshape(NKV, HD, D)

    in_maps = []
    for core in range(8):
        b, g = core // 4, core % 4
        xT = np.ascontiguousarray(x[b].T)                              # [D, L]
        xh_f = (xT.reshape(NKD, 128, TC, TCW).transpose(2, 1, 0, 3)
                .reshape(TC, 128, NKD * TCW))
        xh = xh_f.astype(F16)
        xh8 = (np.clip(xh_f, -240, 240)
               .reshape(TC, 128, NKD // 2, 2, TCW).astype(E4))

        rows = []
        for p in range(4):
            rows.append(wq_part[8 * g + p])
            rows.append(wq_part[8 * g + 4 + p])
        rows.append(wk_part[2 * g]); rows.append(wk_part[2 * g + 1])
        rows.append(wv_part[2 * g]); rows.append(wv_part[2 * g + 1])
        Wc = np.concatenate(rows, axis=0)                              # [768, D]
        wq_f = (Wc.reshape(NOT, 128, NKD, 128).transpose(3, 0, 2, 1)
                .reshape(128, NOT * NKD * 128))
        wq = wq_f.astype(F16)
        wq8 = (np.clip(wq_f * 64.0, -240, 240)
               .reshape(128, NOT * NKD // 2, 2, 128).astype(E4))

        cols = np.empty((4, 128), np.int64)
        for kt in range(4):
            cols[kt, :64] = (8 * g + kt) * HD + np.arange(64)
            cols[kt, 64:] = (8 * g + 4 + kt) * HD + np.arange(64)
        Woc = Wo.T[cols.reshape(-1)]                                   # [512, D]
        wo = (Woc.reshape(4, 128, NKD, 128).transpose(1, 0, 2, 3)
              .reshape(128, 4 * NKD * 128)).astype(F16)

        in_maps.append(dict(xh=xh, xh8=xh8, wq=wq, wq8=wq8, wo=wo,
                            ccss=ccss, msk=msk, id2=id2, prm=prm))
    return in_maps


def _get_nc():
    if "nc" not in _cached:
        _cached["nc"] = _build_nc()
    return _cached["nc"]


def run_sharded(x, Wqkv, Wo, trace=False):
    """Run on 8 cores; returns (out [B,L,D] float32, BassKernelResults)."""
    from concourse.bass_utils import run_bass_kernel_spmd
    nc = _get_nc()
    in_maps = _host_prep(np.asarray(x, np.float32), np.asarray(Wqkv, np.float32),
                         np.asarray(Wo, np.float32))
    res = run_bass_kernel_spmd(nc, in_maps, list(range(8)), trace=trace)
    out = np.zeros((B, L, D), np.float64)
    for core in range(8):
        b = core // 4
        P = res.results[core]["outp"].transpose(1, 2, 0, 3).reshape(D, L)
        out[b] += P.T.astype(np.float64)
    return out.astype(np.float32), res


def kernel(x, Wqkv, Wo):
    out, _ = run_sharded(x, Wqkv, Wo, trace=False)
    return out
